# revision 29
# baseline (speedup 1.0000x reference)
"""Causal multi-head attention on 8 Trainium2 NeuronCores (Bass/Tile).

Problem: Q,K,V [B=2, h=16, S=2048, d=64] fp32; out = softmax(QK^T/8, causal) V.

Sharding: B*h = 32 heads split 4-per-core across 8 cores (head-parallel);
each core computes full causal attention for its 4 heads.

Schedule (vs. the 185us baseline): the PE program is software-pipelined
with skew 2 over a flat global (head, chunk, pair) list so the PE never
waits on softmax: ..., QK(i), PV(i-2), QK(i+1), PV(i-1), ... Keeping the PE
continuously busy also keeps it at the 2.4GHz pstate (an idle PE throttles
to 1.2GHz, which is where most of the baseline's time went).

Engine split: ACT runs ONLY the softmax exp (one instruction per k-tile
pair). DVE handles all PSUM->SBUF copies, diagonal-block causal masking,
and the output normalize. GPSIMD issues the (casting) input DMAs. SP issues
Q^T row-dup + batched output stores.

QK side runs fp32r (SWDGE casting loads, PE transpose-mode); the P/V side
runs bf16: exp writes bf16 P^T directly, V' = [V | 1] is DMA-cast to bf16,
so the PV matmuls take 1 cycle/row at any crop width (fp32r would pay 4x
below 256-wide). Causal masking: matmuls compute block-cropped ranges only;
in-diagonal-block upper triangles are zeroed AFTER exp by one bf16 DVE
multiply per diagonal pair against a 0/1 triangle constant, via a
stride-640 [128, 2, 128] view covering both diagonal tiles of the pair.

Optional: FEXP_PATTERN routes some pairs' exp to DVE as a Schraudolph
fast-exp (tensor_scalar mult+add -> int16, bitcast bf16; ~3% element
error). Off by default - enable only if ACT binds and measured rel err
allows.

Per-head layout:
  - Q,K loaded [128, 16*64] fp32->fp32r via SWDGE; V' [128, 16*65] bf16.
  - PE transpose-mode: Q -> Q^T [64, 2048] fp32r (+SP DMA row-dup to
    64:128), K -> K^T stacked pairs [128, 8*128] fp32r; PSUM->SBUF copies
    on DVE.
  - Pair (c, t): S^T [128, 1024] PSUM (two 64-contraction matmuls, min-256
    crops), ACT exp -> P^T bf16, diag mask, PV: O'^T [65, 512] += V'_j^T @
    P^T_j (row 64 = softmax denominator l).
  - Per chunk: O'^T -> SBUF bf16 (DVE), PE transpose to [128, 4*96] PSUM,
    one batched reciprocal + one broadcast multiply (DVE), one batched
    output store (SP).
"""

import numpy as np

import concourse.bass as bass
import concourse.bass_utils as _bass_utils
import concourse.mybir as mybir
import concourse.tile as tile
from concourse.bass_utils import run_bass_kernel_spmd
from concourse.tile import add_dep_helper

# NOTE: walrus's --enable-ldw-opt=true was tried to dedup/overlap the
# per-matmul LDWEIGHTS (~53us/core serial) but miscompiles this kernel
# (NaN output) — the flag stays at the default false.

N_CORES = 8
B, H, S, D = 2, 16, 2048, 64
HEADS_PER_CORE = (B * H) // N_CORES  # 4
NT = S // 128           # 16 k/q tiles per head
NCHUNK = S // 512       # 4 q-chunks per head
PAIRS_PER_HEAD = sum(2 * c + 2 for c in range(NCHUNK))  # 20
F32 = mybir.dt.float32
F32R = mybir.dt.float32r
BF16 = mybir.dt.bfloat16
I16 = mybir.dt.int16

# Schraudolph fast-exp (bf16 domain): exp(0.125*s) ~= bitcast_bf16(int16(
# s*K1 + K2)). Used only for pairs selected by FEXP_PATTERN.
FEXP_K1 = float(np.float32(0.125 * 1.4426950408889634 * 128))
FEXP_K2 = float(np.float32((127.0 - 0.04367744) * 128))
# pair-counter predicate: which pairs use DVE fast-exp (empty = all ACT)
FEXP_MOD, FEXP_LT = 5, 0  # ctr % FEXP_MOD < FEXP_LT -> DVE


class SplitDrainTileContext(tile.TileContext):
    """TileContext whose tail drain splits its semaphore waits across
    single-wait SP nops — the TPB CTRL_NO struct holds one wait slot, so
    a drain waiting on >1 proc fails walrus codegen."""

    def _drain_and_barrier(self, tick_clock, wait_clock):
        import bass_rust
        from concourse.vector_clock import ScopedClock

        gc = tick_clock.global_clock
        for i, v in enumerate(list(gc)):
            if v <= 0:
                continue
            c = bass_rust.VectorClock()
            c.require_at_least(i, v)
            nop = self.nc.sync.nop(hint="preDrain", nofuse=True)
            wait_clock.add_sem_waits(nop.ins, ScopedClock({None: c}))
        drain_inst = self.nc.sync.drain()
        wait_clock.add_sem_waits(
            drain_inst.ins, ScopedClock({None: bass_rust.VectorClock()})
        )
        self.nc.all_engine_barrier()
        assert self.sems is not None
        popped = self.nc._tile_sem_poison_stack.pop()
        assert popped is self._sem_poison
        self.nc.clear_and_free_semaphores(list(self.sems.allocated().values()))
        self.nc.all_engine_barrier()


def pe_touch(nc, ap):
    """1-column bf16 ldweights reading `ap` — engine-level PE instruction
    that absorbs a producer's sync wait into the PE engine clock so that
    following 4-byte matmuls need at most one wait (walrus S3_LW limit)."""
    return nc.tensor.ldweights(ap.bitcast(mybir.dt.bfloat16))


def split_waits(nc):
    """Post-pass: every TPB instruction holds exactly ONE sync-wait slot;
    walrus codegen rejects more. Move extra waits onto inserted same-engine
    nofuse nops placed immediately before the instruction."""
    cnt = 0
    for fn in nc.m.functions:
        for bb in fn.blocks:
            lst = bb.instructions
            i = 0
            while i < len(lst):
                ins = lst[i]
                si = ins.sync_info
                if si is not None and si.on_wait and len(si.on_wait) > 1:
                    waits = list(si.on_wait)
                    for w in waits[:-1]:
                        nop = mybir.InstNoOp(name=f"wsplit_{cnt}", ins=[], outs=[])
                        cnt += 1
                        nop.engine = ins.engine
                        nop.bass_nofuse = True
                        nop.sync_info = mybir.SyncInfo(on_wait=[w], on_update=[])
                        lst.insert(i, nop)
                        i += 1
                    si.on_wait = [waits[-1]]
                i += 1
    return cnt


def build_kernel():
    nc = bass.Bass(trn_type="TRN2")
    q_d = nc.dram_tensor("Q", [HEADS_PER_CORE, S, D], F32, kind="ExternalInput")
    k_d = nc.dram_tensor("K", [HEADS_PER_CORE, S, D], F32, kind="ExternalInput")
    v_d = nc.dram_tensor("V", [HEADS_PER_CORE, S, D], F32, kind="ExternalInput")
    o_d = nc.dram_tensor("O", [HEADS_PER_CORE, S, D], F32, kind="ExternalOutput")

    with SplitDrainTileContext(nc) as tc:
        import contextlib

        with contextlib.ExitStack() as ctx:
            consts = ctx.enter_context(tc.tile_pool(name="consts", bufs=1))
            in_pool = ctx.enter_context(tc.tile_pool(name="in", bufs=2))
            v_pool = ctx.enter_context(tc.tile_pool(name="vp", bufs=2))
            qt_pool = ctx.enter_context(tc.tile_pool(name="qt", bufs=2))
            kt_pool = ctx.enter_context(tc.tile_pool(name="kt", bufs=2))
            pt_pool = ctx.enter_context(tc.tile_pool(name="pt", bufs=4))
            otsb_pool = ctx.enter_context(tc.tile_pool(name="otsb", bufs=2))
            ob_pool = ctx.enter_context(tc.tile_pool(name="ob", bufs=2))
            r_pool = ctx.enter_context(tc.tile_pool(name="recip", bufs=4))

            st_ps = ctx.enter_context(tc.tile_pool(name="stps", bufs=3, space="PSUM"))
            ot_ps = ctx.enter_context(tc.tile_pool(name="otps", bufs=1, space="PSUM"))
            stage_ps = ctx.enter_context(tc.tile_pool(name="stage", bufs=1, space="PSUM"))

            # ---- constants ----
            ident_f = consts.tile([128, 128], F32, tag="ident_f")
            nc.gpsimd.memset(ident_f[:], 0.0)
            nc.gpsimd.affine_select(
                out=ident_f[:], in_=ident_f[:],
                compare_op=mybir.AluOpType.not_equal, fill=1.0, base=0,
                pattern=[[-1, 128]], channel_multiplier=1,
            )
            ident_r = consts.tile([128, 128], F32R, tag="ident_r")
            nc.vector.tensor_copy(ident_r[:], ident_f[:])
            ident_b = consts.tile([128, 128], BF16, tag="ident_b")
            nc.vector.tensor_copy(ident_b[:], ident_f[:])
            # 0/1 causal keep-mask for one diagonal block of P^T [k, q]:
            # keep (1.0) where q >= k i.e. f >= p, zero where f < p.
            tmask = consts.tile([128, 128], BF16, tag="tmask")
            nc.gpsimd.memset(tmask[:], 1.0)
            # keep 1.0 where f - p + 1 > 0 i.e. q >= k; fill 0.0 above diag
            nc.gpsimd.affine_select(
                out=tmask[:], in_=tmask[:],
                compare_op=mybir.AluOpType.is_gt, fill=0.0, base=1,
                pattern=[[1, 128]], channel_multiplier=-1,
            )
            t_if = pe_touch(nc, ident_f[0:1, 0:1])
            t_ir = pe_touch(nc, ident_r[0:1, 0:1])
            t_ib = pe_touch(nc, ident_b[0:1, 0:1])
            # PE warm-up: keep the array busy early so the pstate ramps to
            # full clock while the first loads land.
            warm = stage_ps.tile([128, 512], F32, tag="stage")
            for _ in range(24):
                nc.tensor.matmul(
                    warm[:, 0:256],
                    ident_f[:, 0:64].bitcast(mybir.dt.bfloat16),
                    ident_f[:, 0:128].bitcast(mybir.dt.bfloat16),
                    start=True, stop=True,
                )

            # ---- per-head prep pieces ----
            def emit_loads(h):
                qn = in_pool.tile([128, NT * 64], BF16, tag="qn")
                kn = in_pool.tile([128, NT * 64], BF16, tag="kn")
                for half in range(2):
                    nc.gpsimd.dma_start(
                        qn[:].rearrange("p (t d) -> p t d", d=64)[:, 8*half:8*half+8, :],
                        q_d[h].rearrange("(t p) d -> p t d", p=128)[:, 8*half:8*half+8, :],
                    )
                    nc.gpsimd.dma_start(
                        kn[:].rearrange("p (t d) -> p t d", d=64)[:, 8*half:8*half+8, :],
                        k_d[h].rearrange("(t p) d -> p t d", p=128)[:, 8*half:8*half+8, :],
                    )
                vp = v_pool.tile([128, NT * 65], BF16, tag="vp")
                vp3 = vp[:].rearrange("p (t e) -> p t e", e=65)
                nc.gpsimd.dma_start(
                    vp3[:, :, 0:64],
                    v_d[h].rearrange("(t p) d -> p t d", p=128),
                )
                nc.gpsimd.memset(vp3[:, :, 64:65], 1.0)
                qt = qt_pool.tile([128, S], BF16, tag="qt")
                kt = kt_pool.tile([128, 8 * 128], BF16, tag="kt")
                hs = {
                    "qn": qn, "kn": kn, "vp": vp, "qt": qt, "kt": kt,
                    "touch": [pe_touch(nc, qn[0:1, 0:1]),
                              pe_touch(nc, kn[0:1, 0:1]),
                              pe_touch(nc, vp[0:1, 0:1]),
                              pe_touch(nc, vp[0:1, 64:65])],
                    "first_tr": None,
                }
                return hs

            def emit_group(hs, g):
                """g 0..3: Q transpose groups, bf16 PE transpose-mode
                (DVE copy + SP row-dup); g 4..5: K^T stacked pairs via
                four XBAR DMA transposes each — zero PE/DVE cost."""
                if g < 4:
                    stage = stage_ps.tile([128, 512], BF16, tag="stage",
                                          name="stage")
                    for s_i in range(4):
                        b = 4 * g + s_i
                        mm = nc.tensor.transpose(
                            stage[0:64, 128 * s_i:128 * s_i + 128],
                            hs["qn"][:, 64 * b:64 * b + 64],
                            ident_b[0:128, 0:128],
                        )
                        if hs["first_tr"] is None:
                            hs["first_tr"] = mm
                            for t in [t_if, t_ir, t_ib] + hs["touch"]:
                                if t is not None:
                                    add_dep_helper(mm.ins, t.ins, sync=False,
                                                   reason="presync")
                    nc.vector.tensor_copy(
                        hs["qt"][0:64, 512 * g:512 * g + 512],
                        stage[0:64, :],
                    )
                    nc.sync.dma_start(
                        hs["qt"][64:128, 512 * g:512 * g + 512],
                        hs["qt"][0:64, 512 * g:512 * g + 512],
                    )
                else:
                    gg = g - 4
                    for s_i in range(4):
                        pg = 4 * gg + s_i
                        nc.sync.dma_start(
                            hs["kt"][:, 128 * pg:128 * pg + 128],
                            hs["kn"][:, 128 * pg:128 * pg + 128],
                            transpose=True,
                        )
                if g == 5:
                    hs["tq1"] = pe_touch(nc, hs["qt"][0:1, 0:1])
                    hs["tk1"] = pe_touch(nc, hs["kt"][0:1, 0:1])

            # ---- pair ops ----
            exp_ctr = [0]

            def emit_qk(hs, h, c, t, first_of_head):
                qt, kt = hs["qt"], hs["kt"]
                j1, j2 = 2 * t, 2 * t + 1
                cA = 128 * j1 - 512 * c
                cB = 128 * j2 - 512 * c
                a1 = max(0, cA)
                a2 = max(0, cB)
                st = st_ps.tile([128, 1024], F32, tag="st")
                mmA = nc.tensor.matmul(
                    st[:, a1:512],
                    kt[0:64, 128 * t:128 * t + 128],
                    qt[0:64, 512 * c + a1:512 * c + 512],
                    start=True, stop=True,
                )
                if first_of_head:
                    for tt in (hs["tq1"], hs["tk1"]):
                        if tt is not None:
                            add_dep_helper(mmA.ins, tt.ins, sync=False,
                                           reason="presync")
                nc.tensor.matmul(
                    st[:, 512 + a2:1024],
                    kt[64:128, 128 * t:128 * t + 128],
                    qt[64:128, 512 * c + a2:512 * c + 512],
                    start=True, stop=True,
                )

                pt = pt_pool.tile([128, 1024], BF16, tag="pt")
                use_dve = (exp_ctr[0] % FEXP_MOD) < FEXP_LT
                exp_ctr[0] += 1
                if use_dve:
                    nc.vector.tensor_scalar(
                        pt[:, a1:1024].bitcast(I16),
                        st[:, a1:1024],
                        FEXP_K1, FEXP_K2,
                        mybir.AluOpType.mult, mybir.AluOpType.add,
                    )
                else:
                    nc.scalar.activation(
                        pt[:, a1:1024], st[:, a1:1024],
                        mybir.ActivationFunctionType.Exp, scale=0.125,
                    )
                # zero the in-block upper triangles of diagonal tiles: the
                # last two pairs of each chunk hold them, at in-pair col
                # offsets (0, 640) for pair 2c and (256, 896) for pair 2c+1.
                npair = 2 * c + 2
                if t >= npair - 2:
                    off = 0 if t == npair - 2 else 256
                    v8 = pt[:].rearrange("p (i x) -> p i x", x=128)
                    i0 = off // 128
                    dview = v8[:, i0:i0 + 6:5, :]
                    nc.vector.tensor_tensor(
                        dview, dview,
                        tmask[:][:, None, :].broadcast_to([128, 2, 128]),
                        mybir.AluOpType.mult,
                    )
                return {"st": st, "pt": pt}

            def emit_pv(hs, h, c, t, tiles, ot_holder):
                pt, vp = tiles["pt"], hs["vp"]
                npair = 2 * c + 2
                if t == 0:
                    ot_holder["ot"] = ot_ps.tile([65, 512], F32, tag="ot",
                                                 name="ot")
                ot = ot_holder["ot"]
                for half, j in enumerate((2 * t, 2 * t + 1)):
                    vA = max(0, 128 * j - 512 * c)
                    nc.tensor.matmul(
                        ot[:, vA:512],
                        vp[:, 65 * j:65 * j + 65],
                        pt[:, 512 * half + vA:512 * half + 512],
                        start=(t == 0 and half == 0),
                        stop=(t == npair - 1 and half == 1),
                        skip_group_check=True,
                    )

            def emit_out_copy(ot_holder):
                ot = ot_holder["ot"]
                otsb = otsb_pool.tile([65, 512], F32R, tag="otsb")
                nc.vector.tensor_copy(otsb[:], ot[:])
                ot_holder["otsb"] = otsb

            def emit_out(hs, h, c, ot_holder):
                otsb = ot_holder["otsb"]
                oq = stage_ps.tile([128, 384], F32R, tag="stage", name="oq")
                for i in range(4):
                    nc.tensor.transpose(
                        oq[:, 96 * i:96 * i + 96],
                        otsb[0:65, 128 * i:128 * i + 128],
                        ident_r[0:65, 0:96],
                    )
                oq4 = oq[:].bitcast(F32).rearrange("p (i x) -> p i x", x=96)
                rec = r_pool.tile([128, 4], F32, tag="rec")
                nc.vector.reciprocal(rec[:][:, :, None], oq4[:, :, 64:65])
                ob = ob_pool.tile([128, 256], F32, tag="ob")
                nc.vector.tensor_tensor(
                    ob[:].rearrange("p (i x) -> p i x", x=64),
                    oq4[:, :, 0:64],
                    rec[:].broadcast_to([128, 4, 64]),
                    mybir.AluOpType.mult,
                )
                nc.sync.dma_start(
                    o_d[h, 512 * c:512 * c + 512, :].rearrange(
                        "(t p) d -> p t d", p=128),
                    ob[:].rearrange("p (t d) -> p t d", d=64),
                )

            # ---- flat skew-2 pipeline over all (head, chunk, pair) ----
            all_pairs = []
            for h in range(HEADS_PER_CORE):
                for c in range(NCHUNK):
                    for t in range(2 * c + 2):
                        all_pairs.append((h, c, t))

            head_state = [None] * HEADS_PER_CORE
            head_state[0] = emit_loads(0)
            for g in range(6):
                emit_group(head_state[0], g)

            tiles_by_idx = {}
            ot_holders = {}
            out_queue = []  # (due_slot, h, c, holder): PE out-part delayed
            n = len(all_pairs)
            SKEW = 2
            OUT_DELAY = 0

            def run_pv(ip):
                hp, cp, tp = all_pairs[ip]
                key = (hp, cp)
                if key not in ot_holders:
                    ot_holders[key] = {}
                emit_pv(head_state[hp], hp, cp, tp, tiles_by_idx.pop(ip),
                        ot_holders[key])
                if tp == 2 * cp + 1:
                    holder = ot_holders.pop(key)
                    emit_out_copy(holder)
                    out_queue.append([ip + OUT_DELAY, hp, cp, holder])

            def flush_outs(slot):
                while out_queue and out_queue[0][0] <= slot:
                    _, hp, cp, holder = out_queue.pop(0)
                    emit_out(head_state[hp], hp, cp, holder)

            for i, (h, c, t) in enumerate(all_pairs):
                local = i - PAIRS_PER_HEAD * h
                if h + 1 < HEADS_PER_CORE:
                    if local == 10:
                        head_state[h + 1] = emit_loads(h + 1)
                    if 12 <= local <= 17:
                        emit_group(head_state[h + 1], local - 12)
                tiles_by_idx[i] = emit_qk(
                    head_state[h], h, c, t, first_of_head=(local == 0))
                if i >= SKEW:
                    run_pv(i - SKEW)
                    flush_outs(i - SKEW)
            for ip in range(n - SKEW, n):
                run_pv(ip)
                flush_outs(ip)
            flush_outs(10 ** 9)

    split_waits(nc)
    return nc


_CACHED = {}


def kernel(Q: np.ndarray, K: np.ndarray, V: np.ndarray) -> np.ndarray:
    res = _run(Q, K, V, trace=False)
    return res[0]


def _run(Q, K, V, trace=False):
    Qf = np.ascontiguousarray(Q.reshape(B * H, S, D), dtype=np.float32)
    Kf = np.ascontiguousarray(K.reshape(B * H, S, D), dtype=np.float32)
    Vf = np.ascontiguousarray(V.reshape(B * H, S, D), dtype=np.float32)

    in_maps = []
    for c in range(N_CORES):
        sl = slice(c * HEADS_PER_CORE, (c + 1) * HEADS_PER_CORE)
        in_maps.append({
            "Q": np.ascontiguousarray(Qf[sl]),
            "K": np.ascontiguousarray(Kf[sl]),
            "V": np.ascontiguousarray(Vf[sl]),
        })

    if "nc" not in _CACHED:
        _CACHED["nc"] = build_kernel()
    nc = _CACHED["nc"]

    res = run_bass_kernel_spmd(
        nc, in_maps, core_ids=list(range(N_CORES)), trace=trace
    )
    out = np.empty((B * H, S, D), dtype=np.float32)
    for c in range(N_CORES):
        out[c * HEADS_PER_CORE:(c + 1) * HEADS_PER_CORE] = res.results[c]["O"]
    return out.reshape(B, H, S, D), res


# revision 31
# speedup vs baseline: 1.3847x; 1.3847x over previous
"""Causal multi-head attention on 8 Trainium2 NeuronCores (Bass/Tile).

Problem: Q,K,V [B=2, h=16, S=2048, d=64] fp32; out = softmax(QK^T/8, causal) V.

Sharding: B*h = 32 heads split 4-per-core across 8 cores (head-parallel);
each core computes full causal attention for its 4 heads.

Schedule (vs. the 185us baseline): the PE program is software-pipelined
with skew 2 over a flat global (head, chunk, pair) list so the PE never
waits on softmax: ..., QK(i), PV(i-2), QK(i+1), PV(i-1), ... Keeping the PE
continuously busy also keeps it at the 2.4GHz pstate (an idle PE throttles
to 1.2GHz, which is where most of the baseline's time went).

Engine split: ACT runs ONLY the softmax exp (one instruction per k-tile
pair). DVE handles all PSUM->SBUF copies, diagonal-block causal masking,
and the output normalize. GPSIMD issues the (casting) input DMAs. SP issues
Q^T row-dup + batched output stores.

QK side runs fp32r (SWDGE casting loads, PE transpose-mode); the P/V side
runs bf16: exp writes bf16 P^T directly, V' = [V | 1] is DMA-cast to bf16,
so the PV matmuls take 1 cycle/row at any crop width (fp32r would pay 4x
below 256-wide). Causal masking: matmuls compute block-cropped ranges only;
in-diagonal-block upper triangles are zeroed AFTER exp by one bf16 DVE
multiply per diagonal pair against a 0/1 triangle constant, via a
stride-640 [128, 2, 128] view covering both diagonal tiles of the pair.

Optional: FEXP_PATTERN routes some pairs' exp to DVE as a Schraudolph
fast-exp (tensor_scalar mult+add -> int16, bitcast bf16; ~3% element
error). Off by default - enable only if ACT binds and measured rel err
allows.

Per-head layout:
  - Q,K loaded [128, 16*64] fp32->fp32r via SWDGE; V' [128, 16*65] bf16.
  - PE transpose-mode: Q -> Q^T [64, 2048] fp32r (+SP DMA row-dup to
    64:128), K -> K^T stacked pairs [128, 8*128] fp32r; PSUM->SBUF copies
    on DVE.
  - Pair (c, t): S^T [128, 1024] PSUM (two 64-contraction matmuls, min-256
    crops), ACT exp -> P^T bf16, diag mask, PV: O'^T [65, 512] += V'_j^T @
    P^T_j (row 64 = softmax denominator l).
  - Per chunk: O'^T -> SBUF bf16 (DVE), PE transpose to [128, 4*96] PSUM,
    one batched reciprocal + one broadcast multiply (DVE), one batched
    output store (SP).
"""

import numpy as np

import concourse.bass as bass
import concourse.bass_utils as _bass_utils
import concourse.mybir as mybir
import concourse.tile as tile
from concourse.bass_utils import run_bass_kernel_spmd
from concourse.tile import add_dep_helper

# NOTE: walrus's --enable-ldw-opt=true was tried to dedup/overlap the
# per-matmul LDWEIGHTS (~53us/core serial) but miscompiles this kernel
# (NaN output) — the flag stays at the default false.

N_CORES = 8
B, H, S, D = 2, 16, 2048, 64
HEADS_PER_CORE = (B * H) // N_CORES  # 4
NT = S // 128           # 16 k/q tiles per head
NCHUNK = S // 512       # 4 q-chunks per head
PAIRS_PER_HEAD = sum(2 * c + 2 for c in range(NCHUNK))  # 20
F32 = mybir.dt.float32
F32R = mybir.dt.float32r
BF16 = mybir.dt.bfloat16
I16 = mybir.dt.int16

# Schraudolph fast-exp (bf16 domain): exp(0.125*s) ~= bitcast_bf16(int16(
# s*K1 + K2)). Used only for pairs selected by FEXP_PATTERN.
FEXP_K1 = float(np.float32(0.125 * 1.4426950408889634 * 128))
FEXP_K2 = float(np.float32((127.0 - 0.04367744) * 128))
# pair-counter predicate: which pairs use DVE fast-exp (empty = all ACT)
FEXP_MOD, FEXP_LT = 5, 0  # ctr % FEXP_MOD < FEXP_LT -> DVE


class SplitDrainTileContext(tile.TileContext):
    """TileContext whose tail drain splits its semaphore waits across
    single-wait SP nops — the TPB CTRL_NO struct holds one wait slot, so
    a drain waiting on >1 proc fails walrus codegen."""

    def _drain_and_barrier(self, tick_clock, wait_clock):
        import bass_rust
        from concourse.vector_clock import ScopedClock

        gc = tick_clock.global_clock
        for i, v in enumerate(list(gc)):
            if v <= 0:
                continue
            c = bass_rust.VectorClock()
            c.require_at_least(i, v)
            nop = self.nc.sync.nop(hint="preDrain", nofuse=True)
            wait_clock.add_sem_waits(nop.ins, ScopedClock({None: c}))
        drain_inst = self.nc.sync.drain()
        wait_clock.add_sem_waits(
            drain_inst.ins, ScopedClock({None: bass_rust.VectorClock()})
        )
        self.nc.all_engine_barrier()
        assert self.sems is not None
        popped = self.nc._tile_sem_poison_stack.pop()
        assert popped is self._sem_poison
        self.nc.clear_and_free_semaphores(list(self.sems.allocated().values()))
        self.nc.all_engine_barrier()


def pe_touch(nc, ap):
    """1-column bf16 ldweights reading `ap` — engine-level PE instruction
    that absorbs a producer's sync wait into the PE engine clock so that
    following 4-byte matmuls need at most one wait (walrus S3_LW limit)."""
    return nc.tensor.ldweights(ap.bitcast(mybir.dt.bfloat16))


def split_waits(nc):
    """Post-pass: every TPB instruction holds exactly ONE sync-wait slot;
    walrus codegen rejects more. Move extra waits onto inserted same-engine
    nofuse nops placed immediately before the instruction."""
    cnt = 0
    for fn in nc.m.functions:
        for bb in fn.blocks:
            lst = bb.instructions
            i = 0
            while i < len(lst):
                ins = lst[i]
                si = ins.sync_info
                if si is not None and si.on_wait and len(si.on_wait) > 1:
                    waits = list(si.on_wait)
                    for w in waits[:-1]:
                        nop = mybir.InstNoOp(name=f"wsplit_{cnt}", ins=[], outs=[])
                        cnt += 1
                        nop.engine = ins.engine
                        nop.bass_nofuse = True
                        nop.sync_info = mybir.SyncInfo(on_wait=[w], on_update=[])
                        lst.insert(i, nop)
                        i += 1
                    si.on_wait = [waits[-1]]
                i += 1
    return cnt


def build_kernel():
    nc = bass.Bass(trn_type="TRN2")
    q_d = nc.dram_tensor("Q", [HEADS_PER_CORE, S, D], F32, kind="ExternalInput")
    k_d = nc.dram_tensor("K", [HEADS_PER_CORE, S, D], F32, kind="ExternalInput")
    v_d = nc.dram_tensor("V", [HEADS_PER_CORE, S, D], F32, kind="ExternalInput")
    o_d = nc.dram_tensor("O", [HEADS_PER_CORE, S, D], F32, kind="ExternalOutput")

    with SplitDrainTileContext(nc) as tc:
        import contextlib

        with contextlib.ExitStack() as ctx:
            consts = ctx.enter_context(tc.tile_pool(name="consts", bufs=1))
            in_pool = ctx.enter_context(tc.tile_pool(name="in", bufs=2))
            v_pool = ctx.enter_context(tc.tile_pool(name="vp", bufs=2))
            qt_pool = ctx.enter_context(tc.tile_pool(name="qt", bufs=2))
            kt_pool = ctx.enter_context(tc.tile_pool(name="kt", bufs=2))
            pt_pool = ctx.enter_context(tc.tile_pool(name="pt", bufs=4))
            otsb_pool = ctx.enter_context(tc.tile_pool(name="otsb", bufs=2))
            ob_pool = ctx.enter_context(tc.tile_pool(name="ob", bufs=2))
            r_pool = ctx.enter_context(tc.tile_pool(name="recip", bufs=4))

            st_ps = ctx.enter_context(tc.tile_pool(name="stps", bufs=3, space="PSUM"))
            ot_ps = ctx.enter_context(tc.tile_pool(name="otps", bufs=1, space="PSUM"))
            stage_ps = ctx.enter_context(tc.tile_pool(name="stage", bufs=1, space="PSUM"))

            # ---- constants ----
            ident_f = consts.tile([128, 128], F32, tag="ident_f")
            nc.gpsimd.memset(ident_f[:], 0.0)
            nc.gpsimd.affine_select(
                out=ident_f[:], in_=ident_f[:],
                compare_op=mybir.AluOpType.not_equal, fill=1.0, base=0,
                pattern=[[-1, 128]], channel_multiplier=1,
            )
            ident_r = consts.tile([128, 128], F32R, tag="ident_r")
            nc.vector.tensor_copy(ident_r[:], ident_f[:])
            ident_b = consts.tile([128, 128], BF16, tag="ident_b")
            nc.vector.tensor_copy(ident_b[:], ident_f[:])
            # 0/1 causal keep-mask for one diagonal block of P^T [k, q]:
            # keep (1.0) where q >= k i.e. f >= p, zero where f < p.
            tmask = consts.tile([128, 128], BF16, tag="tmask")
            nc.gpsimd.memset(tmask[:], 1.0)
            # keep 1.0 where f - p + 1 > 0 i.e. q >= k; fill 0.0 above diag
            nc.gpsimd.affine_select(
                out=tmask[:], in_=tmask[:],
                compare_op=mybir.AluOpType.is_gt, fill=0.0, base=1,
                pattern=[[1, 128]], channel_multiplier=-1,
            )
            t_if = pe_touch(nc, ident_f[0:1, 0:1])
            t_ir = pe_touch(nc, ident_r[0:1, 0:1])
            t_ib = pe_touch(nc, ident_b[0:1, 0:1])
            # PE warm-up: keep the array busy early so the pstate ramps to
            # full clock while the first loads land.
            warm = stage_ps.tile([128, 512], F32, tag="stage")
            for _ in range(24):
                nc.tensor.matmul(
                    warm[:, 0:256],
                    ident_f[:, 0:64].bitcast(mybir.dt.bfloat16),
                    ident_f[:, 0:128].bitcast(mybir.dt.bfloat16),
                    start=True, stop=True,
                )

            # ---- per-head prep pieces ----
            def emit_loads(h):
                qn = in_pool.tile([128, NT * 64], BF16, tag="qn")
                kn = in_pool.tile([128, NT * 64], BF16, tag="kn")
                for half in range(2):
                    nc.gpsimd.dma_start(
                        qn[:].rearrange("p (t d) -> p t d", d=64)[:, 8*half:8*half+8, :],
                        q_d[h].rearrange("(t p) d -> p t d", p=128)[:, 8*half:8*half+8, :],
                    )
                    nc.gpsimd.dma_start(
                        kn[:].rearrange("p (t d) -> p t d", d=64)[:, 8*half:8*half+8, :],
                        k_d[h].rearrange("(t p) d -> p t d", p=128)[:, 8*half:8*half+8, :],
                    )
                vp = v_pool.tile([128, NT * 65], BF16, tag="vp")
                vp3 = vp[:].rearrange("p (t e) -> p t e", e=65)
                nc.gpsimd.dma_start(
                    vp3[:, :, 0:64],
                    v_d[h].rearrange("(t p) d -> p t d", p=128),
                )
                nc.gpsimd.memset(vp3[:, :, 64:65], 1.0)
                qt = qt_pool.tile([128, S], BF16, tag="qt")
                kt = kt_pool.tile([128, 8 * 128], BF16, tag="kt")
                hs = {
                    "qn": qn, "kn": kn, "vp": vp, "qt": qt, "kt": kt,
                    "touch": [pe_touch(nc, qn[0:1, 0:1]),
                              pe_touch(nc, kn[0:1, 0:1]),
                              pe_touch(nc, vp[0:1, 0:1]),
                              pe_touch(nc, vp[0:1, 64:65])],
                    "first_tr": None,
                }
                return hs

            def emit_group(hs, g):
                """g 0..3: Q transpose groups, bf16 PE transpose-mode
                (DVE copy + SP row-dup); g 4..5: K^T stacked pairs, bf16
                PE transpose-mode (DVE copy). An XBAR DMA-transpose K was
                tried: each occupies the Sync engine ~1.2us and starves
                the PE at head boundaries — PE transposes are cheaper."""
                if g < 4:
                    stage = stage_ps.tile([128, 512], BF16, tag="stage",
                                          name="stage")
                    for s_i in range(4):
                        b = 4 * g + s_i
                        mm = nc.tensor.transpose(
                            stage[0:64, 128 * s_i:128 * s_i + 128],
                            hs["qn"][:, 64 * b:64 * b + 64],
                            ident_b[0:128, 0:128],
                        )
                        if hs["first_tr"] is None:
                            hs["first_tr"] = mm
                            for t in [t_if, t_ir, t_ib] + hs["touch"]:
                                if t is not None:
                                    add_dep_helper(mm.ins, t.ins, sync=False,
                                                   reason="presync")
                    nc.vector.tensor_copy(
                        hs["qt"][0:64, 512 * g:512 * g + 512],
                        stage[0:64, :],
                    )
                    nc.sync.dma_start(
                        hs["qt"][64:128, 512 * g:512 * g + 512],
                        hs["qt"][0:64, 512 * g:512 * g + 512],
                    )
                else:
                    gg = g - 4
                    stage = stage_ps.tile([128, 512], BF16, tag="stage",
                                          name="stage")
                    for s_i in range(4):
                        t_i = 4 * gg + s_i
                        nc.tensor.transpose(
                            stage[:, 128 * s_i:128 * s_i + 128],
                            hs["kn"][:, 128 * t_i:128 * t_i + 128],
                            ident_b[0:128, 0:128],
                        )
                    nc.vector.tensor_copy(
                        hs["kt"][:, 512 * gg:512 * gg + 512], stage[:, :]
                    )
                if g == 5:
                    hs["tq1"] = pe_touch(nc, hs["qt"][0:1, 0:1])
                    hs["tk1"] = pe_touch(nc, hs["kt"][0:1, 0:1])

            # ---- pair ops ----
            exp_ctr = [0]

            def emit_qk(hs, h, c, t, first_of_head):
                qt, kt = hs["qt"], hs["kt"]
                j1, j2 = 2 * t, 2 * t + 1
                cA = 128 * j1 - 512 * c
                cB = 128 * j2 - 512 * c
                a1 = max(0, cA)
                a2 = max(0, cB)
                st = st_ps.tile([128, 1024], F32, tag="st")
                mmA = nc.tensor.matmul(
                    st[:, a1:512],
                    kt[0:64, 128 * t:128 * t + 128],
                    qt[0:64, 512 * c + a1:512 * c + 512],
                    start=True, stop=True,
                )
                if first_of_head:
                    for tt in (hs["tq1"], hs["tk1"]):
                        if tt is not None:
                            add_dep_helper(mmA.ins, tt.ins, sync=False,
                                           reason="presync")
                nc.tensor.matmul(
                    st[:, 512 + a2:1024],
                    kt[64:128, 128 * t:128 * t + 128],
                    qt[64:128, 512 * c + a2:512 * c + 512],
                    start=True, stop=True,
                )

                pt = pt_pool.tile([128, 1024], BF16, tag="pt")
                use_dve = (exp_ctr[0] % FEXP_MOD) < FEXP_LT
                exp_ctr[0] += 1
                if use_dve:
                    nc.vector.tensor_scalar(
                        pt[:, a1:1024].bitcast(I16),
                        st[:, a1:1024],
                        FEXP_K1, FEXP_K2,
                        mybir.AluOpType.mult, mybir.AluOpType.add,
                    )
                else:
                    nc.scalar.activation(
                        pt[:, a1:1024], st[:, a1:1024],
                        mybir.ActivationFunctionType.Exp, scale=0.125,
                    )
                # zero the in-block upper triangles of diagonal tiles: the
                # last two pairs of each chunk hold them, at in-pair col
                # offsets (0, 640) for pair 2c and (256, 896) for pair 2c+1.
                npair = 2 * c + 2
                if t >= npair - 2:
                    off = 0 if t == npair - 2 else 256
                    v8 = pt[:].rearrange("p (i x) -> p i x", x=128)
                    i0 = off // 128
                    dview = v8[:, i0:i0 + 6:5, :]
                    nc.vector.tensor_tensor(
                        dview, dview,
                        tmask[:][:, None, :].broadcast_to([128, 2, 128]),
                        mybir.AluOpType.mult,
                    )
                return {"st": st, "pt": pt}

            def emit_pv(hs, h, c, t, tiles, ot_holder):
                pt, vp = tiles["pt"], hs["vp"]
                npair = 2 * c + 2
                if t == 0:
                    ot_holder["ot"] = ot_ps.tile([65, 512], F32, tag="ot",
                                                 name="ot")
                ot = ot_holder["ot"]
                for half, j in enumerate((2 * t, 2 * t + 1)):
                    vA = max(0, 128 * j - 512 * c)
                    nc.tensor.matmul(
                        ot[:, vA:512],
                        vp[:, 65 * j:65 * j + 65],
                        pt[:, 512 * half + vA:512 * half + 512],
                        start=(t == 0 and half == 0),
                        stop=(t == npair - 1 and half == 1),
                        skip_group_check=True,
                    )

            def emit_out_copy(ot_holder):
                ot = ot_holder["ot"]
                otsb = otsb_pool.tile([65, 512], F32R, tag="otsb")
                nc.vector.tensor_copy(otsb[:], ot[:])
                ot_holder["otsb"] = otsb

            def emit_out(hs, h, c, ot_holder):
                otsb = ot_holder["otsb"]
                oq = stage_ps.tile([128, 384], F32R, tag="stage", name="oq")
                for i in range(4):
                    nc.tensor.transpose(
                        oq[:, 96 * i:96 * i + 96],
                        otsb[0:65, 128 * i:128 * i + 128],
                        ident_r[0:65, 0:96],
                    )
                oq4 = oq[:].bitcast(F32).rearrange("p (i x) -> p i x", x=96)
                rec = r_pool.tile([128, 4], F32, tag="rec")
                nc.vector.reciprocal(rec[:][:, :, None], oq4[:, :, 64:65])
                ob = ob_pool.tile([128, 256], F32, tag="ob")
                nc.vector.tensor_tensor(
                    ob[:].rearrange("p (i x) -> p i x", x=64),
                    oq4[:, :, 0:64],
                    rec[:].broadcast_to([128, 4, 64]),
                    mybir.AluOpType.mult,
                )
                nc.sync.dma_start(
                    o_d[h, 512 * c:512 * c + 512, :].rearrange(
                        "(t p) d -> p t d", p=128),
                    ob[:].rearrange("p (t d) -> p t d", d=64),
                )

            # ---- flat skew-2 pipeline over all (head, chunk, pair) ----
            all_pairs = []
            for h in range(HEADS_PER_CORE):
                for c in range(NCHUNK):
                    for t in range(2 * c + 2):
                        all_pairs.append((h, c, t))

            head_state = [None] * HEADS_PER_CORE
            head_state[0] = emit_loads(0)
            for g in range(6):
                emit_group(head_state[0], g)

            tiles_by_idx = {}
            ot_holders = {}
            out_queue = []  # (due_slot, h, c, holder): PE out-part delayed
            n = len(all_pairs)
            SKEW = 2
            OUT_DELAY = 0

            def run_pv(ip):
                hp, cp, tp = all_pairs[ip]
                key = (hp, cp)
                if key not in ot_holders:
                    ot_holders[key] = {}
                emit_pv(head_state[hp], hp, cp, tp, tiles_by_idx.pop(ip),
                        ot_holders[key])
                if tp == 2 * cp + 1:
                    holder = ot_holders.pop(key)
                    emit_out_copy(holder)
                    out_queue.append([ip + OUT_DELAY, hp, cp, holder])

            def flush_outs(slot):
                while out_queue and out_queue[0][0] <= slot:
                    _, hp, cp, holder = out_queue.pop(0)
                    emit_out(head_state[hp], hp, cp, holder)

            for i, (h, c, t) in enumerate(all_pairs):
                local = i - PAIRS_PER_HEAD * h
                if h + 1 < HEADS_PER_CORE:
                    if local == 10:
                        head_state[h + 1] = emit_loads(h + 1)
                    if 12 <= local <= 17:
                        emit_group(head_state[h + 1], local - 12)
                tiles_by_idx[i] = emit_qk(
                    head_state[h], h, c, t, first_of_head=(local == 0))
                if i >= SKEW:
                    run_pv(i - SKEW)
                    flush_outs(i - SKEW)
            for ip in range(n - SKEW, n):
                run_pv(ip)
                flush_outs(ip)
            flush_outs(10 ** 9)

    split_waits(nc)
    return nc


_CACHED = {}


def kernel(Q: np.ndarray, K: np.ndarray, V: np.ndarray) -> np.ndarray:
    res = _run(Q, K, V, trace=False)
    return res[0]


def _run(Q, K, V, trace=False):
    Qf = np.ascontiguousarray(Q.reshape(B * H, S, D), dtype=np.float32)
    Kf = np.ascontiguousarray(K.reshape(B * H, S, D), dtype=np.float32)
    Vf = np.ascontiguousarray(V.reshape(B * H, S, D), dtype=np.float32)

    in_maps = []
    for c in range(N_CORES):
        sl = slice(c * HEADS_PER_CORE, (c + 1) * HEADS_PER_CORE)
        in_maps.append({
            "Q": np.ascontiguousarray(Qf[sl]),
            "K": np.ascontiguousarray(Kf[sl]),
            "V": np.ascontiguousarray(Vf[sl]),
        })

    if "nc" not in _CACHED:
        _CACHED["nc"] = build_kernel()
    nc = _CACHED["nc"]

    res = run_bass_kernel_spmd(
        nc, in_maps, core_ids=list(range(N_CORES)), trace=trace
    )
    out = np.empty((B * H, S, D), dtype=np.float32)
    for c in range(N_CORES):
        out[c * HEADS_PER_CORE:(c + 1) * HEADS_PER_CORE] = res.results[c]["O"]
    return out.reshape(B, H, S, D), res


# revision 36
# speedup vs baseline: 1.3997x; 1.0109x over previous
"""Causal multi-head attention on 8 Trainium2 NeuronCores (Bass/Tile).

Problem: Q,K,V [B=2, h=16, S=2048, d=64] fp32; out = softmax(QK^T/8, causal) V.

Sharding: B*h = 32 heads split 4-per-core across 8 cores (head-parallel);
each core computes full causal attention for its 4 heads.

Schedule (vs. the 185us baseline): the PE program is software-pipelined
with skew 2 over a flat global (head, chunk, pair) list so the PE never
waits on softmax: ..., QK(i), PV(i-2), QK(i+1), PV(i-1), ... Keeping the PE
continuously busy also keeps it at the 2.4GHz pstate (an idle PE throttles
to 1.2GHz, which is where most of the baseline's time went).

Engine split: ACT runs ONLY the softmax exp (one instruction per k-tile
pair). DVE handles all PSUM->SBUF copies, diagonal-block causal masking,
and the output normalize. GPSIMD issues the (casting) input DMAs. SP issues
Q^T row-dup + batched output stores.

QK side runs fp32r (SWDGE casting loads, PE transpose-mode); the P/V side
runs bf16: exp writes bf16 P^T directly, V' = [V | 1] is DMA-cast to bf16,
so the PV matmuls take 1 cycle/row at any crop width (fp32r would pay 4x
below 256-wide). Causal masking: matmuls compute block-cropped ranges only;
in-diagonal-block upper triangles are zeroed AFTER exp by one bf16 DVE
multiply per diagonal pair against a 0/1 triangle constant, via a
stride-640 [128, 2, 128] view covering both diagonal tiles of the pair.

Optional: FEXP_PATTERN routes some pairs' exp to DVE as a Schraudolph
fast-exp (tensor_scalar mult+add -> int16, bitcast bf16; ~3% element
error). Off by default - enable only if ACT binds and measured rel err
allows.

Per-head layout:
  - Q,K loaded [128, 16*64] fp32->fp32r via SWDGE; V' [128, 16*65] bf16.
  - PE transpose-mode: Q -> Q^T [64, 2048] fp32r (+SP DMA row-dup to
    64:128), K -> K^T stacked pairs [128, 8*128] fp32r; PSUM->SBUF copies
    on DVE.
  - Pair (c, t): S^T [128, 1024] PSUM (two 64-contraction matmuls, min-256
    crops), ACT exp -> P^T bf16, diag mask, PV: O'^T [65, 512] += V'_j^T @
    P^T_j (row 64 = softmax denominator l).
  - Per chunk: O'^T -> SBUF bf16 (DVE), PE transpose to [128, 4*96] PSUM,
    one batched reciprocal + one broadcast multiply (DVE), one batched
    output store (SP).
"""

import numpy as np

import concourse.bass as bass
import concourse.bass_utils as _bass_utils
import concourse.mybir as mybir
import concourse.tile as tile
from concourse.bass_utils import run_bass_kernel_spmd
from concourse.tile import add_dep_helper

# NOTE: walrus's --enable-ldw-opt=true was tried to dedup/overlap the
# per-matmul LDWEIGHTS (~53us/core serial) but miscompiles this kernel
# (NaN output) — the flag stays at the default false.

N_CORES = 8
B, H, S, D = 2, 16, 2048, 64
HEADS_PER_CORE = (B * H) // N_CORES  # 4
NT = S // 128           # 16 k/q tiles per head
NCHUNK = S // 512       # 4 q-chunks per head
PAIRS_PER_HEAD = sum(2 * c + 2 for c in range(NCHUNK))  # 20
F32 = mybir.dt.float32
F32R = mybir.dt.float32r
BF16 = mybir.dt.bfloat16
I16 = mybir.dt.int16

# Schraudolph fast-exp (bf16 domain): exp(0.125*s) ~= bitcast_bf16(int16(
# s*K1 + K2)). Used only for pairs selected by FEXP_PATTERN.
FEXP_K1 = float(np.float32(0.125 * 1.4426950408889634 * 128))
FEXP_K2 = float(np.float32((127.0 - 0.04367744) * 128))
# pair-counter predicate: which pairs use DVE fast-exp (empty = all ACT)
FEXP_MOD, FEXP_LT = 5, 0  # ctr % FEXP_MOD < FEXP_LT -> DVE


class SplitDrainTileContext(tile.TileContext):
    """TileContext whose tail drain splits its semaphore waits across
    single-wait SP nops — the TPB CTRL_NO struct holds one wait slot, so
    a drain waiting on >1 proc fails walrus codegen."""

    def _drain_and_barrier(self, tick_clock, wait_clock):
        import bass_rust
        from concourse.vector_clock import ScopedClock

        gc = tick_clock.global_clock
        for i, v in enumerate(list(gc)):
            if v <= 0:
                continue
            c = bass_rust.VectorClock()
            c.require_at_least(i, v)
            nop = self.nc.sync.nop(hint="preDrain", nofuse=True)
            wait_clock.add_sem_waits(nop.ins, ScopedClock({None: c}))
        drain_inst = self.nc.sync.drain()
        wait_clock.add_sem_waits(
            drain_inst.ins, ScopedClock({None: bass_rust.VectorClock()})
        )
        self.nc.all_engine_barrier()
        assert self.sems is not None
        popped = self.nc._tile_sem_poison_stack.pop()
        assert popped is self._sem_poison
        self.nc.clear_and_free_semaphores(list(self.sems.allocated().values()))
        self.nc.all_engine_barrier()


def pe_touch(nc, ap):
    """1-column bf16 ldweights reading `ap` — engine-level PE instruction
    that absorbs a producer's sync wait into the PE engine clock so that
    following 4-byte matmuls need at most one wait (walrus S3_LW limit)."""
    return nc.tensor.ldweights(ap.bitcast(mybir.dt.bfloat16))


def split_waits(nc):
    """Post-pass: every TPB instruction holds exactly ONE sync-wait slot;
    walrus codegen rejects more. Move extra waits onto inserted same-engine
    nofuse nops placed immediately before the instruction."""
    cnt = 0
    for fn in nc.m.functions:
        for bb in fn.blocks:
            lst = bb.instructions
            i = 0
            while i < len(lst):
                ins = lst[i]
                si = ins.sync_info
                if si is not None and si.on_wait and len(si.on_wait) > 1:
                    waits = list(si.on_wait)
                    for w in waits[:-1]:
                        nop = mybir.InstNoOp(name=f"wsplit_{cnt}", ins=[], outs=[])
                        cnt += 1
                        nop.engine = ins.engine
                        nop.bass_nofuse = True
                        nop.sync_info = mybir.SyncInfo(on_wait=[w], on_update=[])
                        lst.insert(i, nop)
                        i += 1
                    si.on_wait = [waits[-1]]
                i += 1
    return cnt


def build_kernel():
    nc = bass.Bass(trn_type="TRN2")
    q_d = nc.dram_tensor("Q", [HEADS_PER_CORE, S, D], F32, kind="ExternalInput")
    k_d = nc.dram_tensor("K", [HEADS_PER_CORE, S, D], F32, kind="ExternalInput")
    v_d = nc.dram_tensor("V", [HEADS_PER_CORE, S, D], F32, kind="ExternalInput")
    o_d = nc.dram_tensor("O", [HEADS_PER_CORE, S, D], F32, kind="ExternalOutput")

    with SplitDrainTileContext(nc) as tc:
        import contextlib

        with contextlib.ExitStack() as ctx:
            consts = ctx.enter_context(tc.tile_pool(name="consts", bufs=1))
            in_pool = ctx.enter_context(tc.tile_pool(name="in", bufs=2))
            v_pool = ctx.enter_context(tc.tile_pool(name="vp", bufs=2))
            qt_pool = ctx.enter_context(tc.tile_pool(name="qt", bufs=2))
            kt_pool = ctx.enter_context(tc.tile_pool(name="kt", bufs=2))
            pt_pool = ctx.enter_context(tc.tile_pool(name="pt", bufs=4))
            otsb_pool = ctx.enter_context(tc.tile_pool(name="otsb", bufs=2))
            ob_pool = ctx.enter_context(tc.tile_pool(name="ob", bufs=2))
            r_pool = ctx.enter_context(tc.tile_pool(name="recip", bufs=4))

            st_ps = ctx.enter_context(tc.tile_pool(name="stps", bufs=3, space="PSUM"))
            ot_ps = ctx.enter_context(tc.tile_pool(name="otps", bufs=1, space="PSUM"))
            stage_ps = ctx.enter_context(tc.tile_pool(name="stage", bufs=1, space="PSUM"))

            # ---- constants ----
            ident_f = consts.tile([128, 128], F32, tag="ident_f")
            nc.gpsimd.memset(ident_f[:], 0.0)
            nc.gpsimd.affine_select(
                out=ident_f[:], in_=ident_f[:],
                compare_op=mybir.AluOpType.not_equal, fill=1.0, base=0,
                pattern=[[-1, 128]], channel_multiplier=1,
            )
            ident_r = consts.tile([128, 128], F32R, tag="ident_r")
            nc.vector.tensor_copy(ident_r[:], ident_f[:])
            ident_b = consts.tile([128, 128], BF16, tag="ident_b")
            nc.vector.tensor_copy(ident_b[:], ident_f[:])
            # 0/1 causal keep-mask for one diagonal block of P^T [k, q]:
            # keep (1.0) where q >= k i.e. f >= p, zero where f < p.
            tmask = consts.tile([128, 128], BF16, tag="tmask")
            nc.gpsimd.memset(tmask[:], 1.0)
            # keep 1.0 where f - p + 1 > 0 i.e. q >= k; fill 0.0 above diag
            nc.gpsimd.affine_select(
                out=tmask[:], in_=tmask[:],
                compare_op=mybir.AluOpType.is_gt, fill=0.0, base=1,
                pattern=[[1, 128]], channel_multiplier=-1,
            )
            t_if = pe_touch(nc, ident_f[0:1, 0:1])
            t_ir = pe_touch(nc, ident_r[0:1, 0:1])
            t_ib = pe_touch(nc, ident_b[0:1, 0:1])
            # PE warm-up: keep the array busy early so the pstate ramps to
            # full clock while the first loads land.
            warm = stage_ps.tile([128, 512], F32, tag="stage")
            for _ in range(24):
                nc.tensor.matmul(
                    warm[:, 0:256],
                    ident_f[:, 0:64].bitcast(mybir.dt.bfloat16),
                    ident_f[:, 0:128].bitcast(mybir.dt.bfloat16),
                    start=True, stop=True,
                )

            # ---- per-head prep pieces ----
            def emit_loads(h):
                qn = in_pool.tile([128, NT * 64], BF16, tag="qn")
                kn = in_pool.tile([128, NT * 64], BF16, tag="kn")
                for half in range(2):
                    nc.gpsimd.dma_start(
                        qn[:].rearrange("p (t d) -> p t d", d=64)[:, 8*half:8*half+8, :],
                        q_d[h].rearrange("(t p) d -> p t d", p=128)[:, 8*half:8*half+8, :],
                    )
                    nc.gpsimd.dma_start(
                        kn[:].rearrange("p (t d) -> p t d", d=64)[:, 8*half:8*half+8, :],
                        k_d[h].rearrange("(t p) d -> p t d", p=128)[:, 8*half:8*half+8, :],
                    )
                vp = v_pool.tile([128, NT * 65], BF16, tag="vp")
                vp3 = vp[:].rearrange("p (t e) -> p t e", e=65)
                nc.gpsimd.dma_start(
                    vp3[:, :, 0:64],
                    v_d[h].rearrange("(t p) d -> p t d", p=128),
                )
                nc.gpsimd.memset(vp3[:, :, 64:65], 1.0)
                qt = qt_pool.tile([128, S], BF16, tag="qt")
                kt = kt_pool.tile([128, 8 * 128], BF16, tag="kt")
                hs = {
                    "qn": qn, "kn": kn, "vp": vp, "qt": qt, "kt": kt,
                    "touch": [pe_touch(nc, qn[0:1, 0:1]),
                              pe_touch(nc, kn[0:1, 0:1]),
                              pe_touch(nc, vp[0:1, 0:1]),
                              pe_touch(nc, vp[0:1, 64:65])],
                    "first_tr": None,
                }
                return hs

            def emit_group(hs, g):
                """g 0..3: Q transpose groups, bf16 PE transpose-mode
                (DVE copy + SP row-dup); g 4..5: K^T stacked pairs, bf16
                PE transpose-mode (DVE copy). An XBAR DMA-transpose K was
                tried: each occupies the Sync engine ~1.2us and starves
                the PE at head boundaries — PE transposes are cheaper."""
                if g < 4:
                    stage = stage_ps.tile([128, 512], BF16, tag="stage",
                                          name="stage")
                    for s_i in range(4):
                        b = 4 * g + s_i
                        mm = nc.tensor.transpose(
                            stage[0:64, 128 * s_i:128 * s_i + 128],
                            hs["qn"][:, 64 * b:64 * b + 64],
                            ident_b[0:128, 0:128],
                        )
                        if hs["first_tr"] is None:
                            hs["first_tr"] = mm
                            for t in [t_if, t_ir, t_ib] + hs["touch"]:
                                if t is not None:
                                    add_dep_helper(mm.ins, t.ins, sync=False,
                                                   reason="presync")
                    nc.vector.tensor_copy(
                        hs["qt"][0:64, 512 * g:512 * g + 512],
                        stage[0:64, :],
                    )
                    nc.sync.dma_start(
                        hs["qt"][64:128, 512 * g:512 * g + 512],
                        hs["qt"][0:64, 512 * g:512 * g + 512],
                    )
                else:
                    gg = g - 4
                    stage = stage_ps.tile([128, 512], BF16, tag="stage",
                                          name="stage")
                    for s_i in range(4):
                        t_i = 4 * gg + s_i
                        mm = nc.tensor.transpose(
                            stage[:, 128 * s_i:128 * s_i + 128],
                            hs["kn"][:, 128 * t_i:128 * t_i + 128],
                            ident_b[0:128, 0:128],
                        )
                        if hs["first_tr"] is None:
                            hs["first_tr"] = mm
                            for t in [t_if, t_ir, t_ib] + hs["touch"]:
                                if t is not None:
                                    add_dep_helper(mm.ins, t.ins, sync=False,
                                                   reason="presync")
                    nc.vector.tensor_copy(
                        hs["kt"][:, 512 * gg:512 * gg + 512], stage[:, :]
                    )
                if g == 3:
                    hs["tq1"] = pe_touch(nc, hs["qt"][0:1, 0:1])
                    hs["tk1"] = pe_touch(nc, hs["kt"][0:1, 0:1])

            # ---- pair ops ----
            exp_ctr = [0]

            def emit_qk(hs, h, c, t, first_of_head):
                qt, kt = hs["qt"], hs["kt"]
                j1, j2 = 2 * t, 2 * t + 1
                cA = 128 * j1 - 512 * c
                cB = 128 * j2 - 512 * c
                a1 = max(0, cA)
                a2 = max(0, cB)
                st = st_ps.tile([128, 1024], F32, tag="st")
                mmA = nc.tensor.matmul(
                    st[:, a1:512],
                    kt[0:64, 128 * t:128 * t + 128],
                    qt[0:64, 512 * c + a1:512 * c + 512],
                    start=True, stop=True,
                )
                if first_of_head:
                    for tt in (hs.get("tq1"), hs.get("tk1")):
                        if tt is not None:
                            add_dep_helper(mmA.ins, tt.ins, sync=False,
                                           reason="presync")
                nc.tensor.matmul(
                    st[:, 512 + a2:1024],
                    kt[64:128, 128 * t:128 * t + 128],
                    qt[64:128, 512 * c + a2:512 * c + 512],
                    start=True, stop=True,
                )

                pt = pt_pool.tile([128, 1024], BF16, tag="pt")
                use_dve = (exp_ctr[0] % FEXP_MOD) < FEXP_LT
                exp_ctr[0] += 1
                if use_dve:
                    nc.vector.tensor_scalar(
                        pt[:, a1:1024].bitcast(I16),
                        st[:, a1:1024],
                        FEXP_K1, FEXP_K2,
                        mybir.AluOpType.mult, mybir.AluOpType.add,
                    )
                else:
                    nc.scalar.activation(
                        pt[:, a1:1024], st[:, a1:1024],
                        mybir.ActivationFunctionType.Exp, scale=0.125,
                    )
                # zero the in-block upper triangles of diagonal tiles: the
                # last two pairs of each chunk hold them, at in-pair col
                # offsets (0, 640) for pair 2c and (256, 896) for pair 2c+1.
                npair = 2 * c + 2
                if t >= npair - 2:
                    off = 0 if t == npair - 2 else 256
                    v8 = pt[:].rearrange("p (i x) -> p i x", x=128)
                    i0 = off // 128
                    dview = v8[:, i0:i0 + 6:5, :]
                    nc.vector.tensor_tensor(
                        dview, dview,
                        tmask[:][:, None, :].broadcast_to([128, 2, 128]),
                        mybir.AluOpType.mult,
                    )
                return {"st": st, "pt": pt}

            def emit_pv(hs, h, c, t, tiles, ot_holder):
                pt, vp = tiles["pt"], hs["vp"]
                npair = 2 * c + 2
                if t == 0:
                    ot_holder["ot"] = ot_ps.tile([65, 512], F32, tag="ot",
                                                 name="ot")
                ot = ot_holder["ot"]
                for half, j in enumerate((2 * t, 2 * t + 1)):
                    vA = max(0, 128 * j - 512 * c)
                    nc.tensor.matmul(
                        ot[:, vA:512],
                        vp[:, 65 * j:65 * j + 65],
                        pt[:, 512 * half + vA:512 * half + 512],
                        start=(t == 0 and half == 0),
                        stop=(t == npair - 1 and half == 1),
                        skip_group_check=True,
                    )

            def emit_out_copy(ot_holder):
                ot = ot_holder["ot"]
                otsb = otsb_pool.tile([65, 512], F32R, tag="otsb")
                nc.vector.tensor_copy(otsb[:], ot[:])
                ot_holder["otsb"] = otsb

            def emit_out(hs, h, c, ot_holder):
                otsb = ot_holder["otsb"]
                oq = stage_ps.tile([128, 384], F32R, tag="stage", name="oq")
                for i in range(4):
                    nc.tensor.transpose(
                        oq[:, 96 * i:96 * i + 96],
                        otsb[0:65, 128 * i:128 * i + 128],
                        ident_r[0:65, 0:96],
                    )
                oq4 = oq[:].bitcast(F32).rearrange("p (i x) -> p i x", x=96)
                rec = r_pool.tile([128, 4], F32, tag="rec")
                nc.vector.reciprocal(rec[:][:, :, None], oq4[:, :, 64:65])
                ob = ob_pool.tile([128, 256], F32, tag="ob")
                nc.vector.tensor_tensor(
                    ob[:].rearrange("p (i x) -> p i x", x=64),
                    oq4[:, :, 0:64],
                    rec[:].broadcast_to([128, 4, 64]),
                    mybir.AluOpType.mult,
                )
                nc.sync.dma_start(
                    o_d[h, 512 * c:512 * c + 512, :].rearrange(
                        "(t p) d -> p t d", p=128),
                    ob[:].rearrange("p (t d) -> p t d", d=64),
                )

            # ---- flat skew-2 pipeline over all (head, chunk, pair) ----
            all_pairs = []
            for h in range(HEADS_PER_CORE):
                for c in range(NCHUNK):
                    for t in range(2 * c + 2):
                        all_pairs.append((h, c, t))

            # K^T groups first so kt is ready when a head's first QK fires;
            # head 0 interleaves its last Q groups into its first pairs.
            PREP_ORDER = [4, 5, 0, 1, 2, 3]
            head_state = [None] * HEADS_PER_CORE
            head_state[0] = emit_loads(0)
            for g in PREP_ORDER[:3]:
                emit_group(head_state[0], g)

            tiles_by_idx = {}
            ot_holders = {}
            out_queue = []  # (due_slot, h, c, holder): PE out-part delayed
            n = len(all_pairs)
            SKEW = 2
            OUT_DELAY = 0

            def run_pv(ip):
                hp, cp, tp = all_pairs[ip]
                key = (hp, cp)
                if key not in ot_holders:
                    ot_holders[key] = {}
                emit_pv(head_state[hp], hp, cp, tp, tiles_by_idx.pop(ip),
                        ot_holders[key])
                if tp == 2 * cp + 1:
                    holder = ot_holders.pop(key)
                    emit_out_copy(holder)
                    out_queue.append([ip + OUT_DELAY, hp, cp, holder])

            def flush_outs(slot):
                while out_queue and out_queue[0][0] <= slot:
                    _, hp, cp, holder = out_queue.pop(0)
                    emit_out(head_state[hp], hp, cp, holder)

            for i, (h, c, t) in enumerate(all_pairs):
                local = i - PAIRS_PER_HEAD * h
                if h + 1 < HEADS_PER_CORE:
                    if local == 10:
                        head_state[h + 1] = emit_loads(h + 1)
                    if 12 <= local <= 17:
                        emit_group(head_state[h + 1], PREP_ORDER[local - 12])
                tiles_by_idx[i] = emit_qk(
                    head_state[h], h, c, t, first_of_head=(local == 0))
                if h == 0 and 0 <= local <= 2:
                    emit_group(head_state[0], PREP_ORDER[3 + local])
                if i >= SKEW:
                    run_pv(i - SKEW)
                    flush_outs(i - SKEW)
            for ip in range(n - SKEW, n):
                run_pv(ip)
                flush_outs(ip)
            flush_outs(10 ** 9)

    split_waits(nc)
    return nc


_CACHED = {}


def kernel(Q: np.ndarray, K: np.ndarray, V: np.ndarray) -> np.ndarray:
    res = _run(Q, K, V, trace=False)
    return res[0]


def _run(Q, K, V, trace=False):
    Qf = np.ascontiguousarray(Q.reshape(B * H, S, D), dtype=np.float32)
    Kf = np.ascontiguousarray(K.reshape(B * H, S, D), dtype=np.float32)
    Vf = np.ascontiguousarray(V.reshape(B * H, S, D), dtype=np.float32)

    in_maps = []
    for c in range(N_CORES):
        sl = slice(c * HEADS_PER_CORE, (c + 1) * HEADS_PER_CORE)
        in_maps.append({
            "Q": np.ascontiguousarray(Qf[sl]),
            "K": np.ascontiguousarray(Kf[sl]),
            "V": np.ascontiguousarray(Vf[sl]),
        })

    if "nc" not in _CACHED:
        _CACHED["nc"] = build_kernel()
    nc = _CACHED["nc"]

    res = run_bass_kernel_spmd(
        nc, in_maps, core_ids=list(range(N_CORES)), trace=trace
    )
    out = np.empty((B * H, S, D), dtype=np.float32)
    for c in range(N_CORES):
        out[c * HEADS_PER_CORE:(c + 1) * HEADS_PER_CORE] = res.results[c]["O"]
    return out.reshape(B, H, S, D), res
